# revision 1
# baseline (speedup 1.0000x reference)
"""Masked self-attention Trainium2 kernel (8 NeuronCores, Bass/Tile).

Problem: B=4, S=2048, D=1024, DK=128 fp32.
  Q = X@Wq + bq; K = X@Wk + bk; V = X@Wv + bv
  scores = Q@K^T / sqrt(DK); masked = scores + tril(ones)*(-1e9)
  out = softmax(masked) @ V

Sharding: core = (batch b = core//2) x (row-half h = core%2). Each core
computes 64 query rows of each of the 16 query tiles of its batch
(rows 128c + 64h + j). All cores run an identical program; per-core
differences are carried entirely in the input data (a column
permutation of X^T and small mask/fix vectors).

Device layouts (all transposed so the PE contracts over partitions):
  X^T [D, S] (host-transposed, per-tile column permuted: own rows first)
  Q^T/K^T [DK, *] = W-chunks(lhsT) x X^T(moving), f32r matmuls
  scores^T [s-chunk 128, q-prefix] = K^T-chunk(lhsT) x Q^T(moving)
  causal skip: chunk c only attends query tiles qi <= c -> contiguous
  q-prefix of width 64*(c+1); single [128,64] mask block on the last
  64 columns (the diagonal tile)
  softmax: exp without max-subtraction (scores are O(1); masked lanes
  underflow to exactly 0); row sums via an M=1 all-ones matmul;
  normalization via exp(-ln(sums)) on ScalarE and an M=1->128 matmul
  broadcast (DVE reciprocal is ~6x slower than the ln/exp pair)
  out^T [DK, 1024] accumulated in PSUM across s-chunks; the globally
  fully-masked last row (2047) is patched via a rank-1 (K=1) matmul
  adding mean(V) with weight from the per-core fix vectors.

  All matmul operands are float16 (11-bit mantissa, ~2.4e-4 rounding --
  the same precision class as the PE's f32r/TF32 mode for this N(0,1)
  data) with fp32 PSUM accumulation. vs f32r this halves the X DMA,
  enables fast weight loads (FWL; fp32-path LDWEIGHTS cannot be
  hidden), and has no small-N throughput penalty. Range is safe: all
  fp16-stored tensors are O(1)..O(100); scores/sums/outputs stay fp32.
  The first weight chunk gets a dedicated small first-wave DMA because
  the DGE queues fair-share HBM bandwidth and gate the first matmul.
"""

import numpy as np

import concourse.bacc as bacc
import concourse.tile as tile
import concourse.mybir as mybir
from concourse.bass_utils import run_bass_kernel_spmd

F32 = mybir.dt.float32
F32R = mybir.dt.float32r
F16 = mybir.dt.float16
BF16 = mybir.dt.bfloat16
AF = mybir.ActivationFunctionType

B, S, D, DK = 4, 2048, 1024, 128
NEG = -1.0e9
NCORES = 8
NBLK = 4          # s-blocks of 512
NCHUNK = 16       # s-chunks of 128
QL = 1024         # local query columns per core (16 tiles x 64)

_cache = {}


def _build():
    nc = bacc.Bacc("TRN2", target_bir_lowering=False, debug=False,
                   num_devices=NCORES)

    xt = nc.dram_tensor("xt", [D, S], F16, kind="ExternalInput")
    wq = nc.dram_tensor("wq", [128, 8, DK], F16, kind="ExternalInput")
    wk = nc.dram_tensor("wk", [128, 8, DK], F16, kind="ExternalInput")
    wv = nc.dram_tensor("wv", [128, 8, DK], F16, kind="ExternalInput")
    bq = nc.dram_tensor("bq", [DK, 1], F32, kind="ExternalInput")
    bk = nc.dram_tensor("bk", [DK, 1], F32, kind="ExternalInput")
    bv = nc.dram_tensor("bv", [DK, 1], F32, kind="ExternalInput")
    maskd = nc.dram_tensor("maskd", [128, 64], F32, kind="ExternalInput")
    zerod = nc.dram_tensor("zerod", [128, 512], F16, kind="ExternalInput")
    onesd = nc.dram_tensor("onesd", [128, 128], F16, kind="ExternalInput")
    idend = nc.dram_tensor("idend", [128, 128], F16, kind="ExternalInput")
    fixod = nc.dram_tensor("fixod", [1, QL], F16, kind="ExternalInput")
    fixsd = nc.dram_tensor("fixsd", [1, QL], F16, kind="ExternalInput")
    outT = nc.dram_tensor("outT", [DK, QL], F32, kind="ExternalOutput")

    with tile.TileContext(nc) as tc:
        with (
            tc.tile_pool(name="consts", bufs=1) as cpool,
            tc.tile_pool(name="xblk", bufs=3) as xpool,
            tc.tile_pool(name="kv", bufs=1) as kvpool,
            tc.tile_pool(name="pt", bufs=3) as ppool,
            tc.tile_pool(name="outp", bufs=1) as opool,
            tc.tile_pool(name="ps_out", bufs=1, space="PSUM") as ps_out_pool,
            tc.tile_pool(name="ps_sums", bufs=1, space="PSUM") as ps_sums_pool,
            tc.tile_pool(name="ps_proj", bufs=2, space="PSUM") as ps_proj_pool,
            tc.tile_pool(name="ps_score", bufs=2, space="PSUM") as ps_score_pool,
        ):
            # ---- weights first (needed by the very first matmul).
            # The first proj matmul (K, dc=0) gates the whole PE stream, so
            # its 64 KiB weight chunk gets a dedicated first DMA: the DGE
            # queues fair-share HBM bandwidth, so a small exclusive first
            # wave completes ~10x sooner than one queued with everything.
            w_sb = {}
            for name, dram in (("k", wk), ("v", wv), ("q", wq)):
                t = cpool.tile([128, 8, DK], F16, tag=f"w{name}")
                if name == "k":
                    nc.scalar.dma_start(out=t[:, 0:1], in_=dram[:, 0:1])
                    nc.scalar.dma_start(out=t[:, 1:8], in_=dram[:, 1:8])
                else:
                    nc.scalar.dma_start(out=t[:], in_=dram[:])
                w_sb[name] = t

            def small_consts():
                b_sb = {}
                for name, dram in (("q", bq), ("k", bk), ("v", bv)):
                    t = cpool.tile([DK, 1], F32, tag=f"b{name}")
                    nc.gpsimd.dma_start(out=t[:], in_=dram[:])
                    b_sb[name] = t
                mask_sb = cpool.tile([128, 64], F32, tag="mask")
                nc.gpsimd.dma_start(out=mask_sb[:], in_=maskd[:])
                zero_sb = cpool.tile([128, 512], F16, tag="zero")
                nc.gpsimd.dma_start(out=zero_sb[:], in_=zerod[:])
                ones_sb = cpool.tile([128, 128], F16, tag="ones")
                nc.gpsimd.dma_start(out=ones_sb[:], in_=onesd[:])
                iden_sb = cpool.tile([128, 128], F16, tag="iden")
                nc.gpsimd.dma_start(out=iden_sb[:], in_=idend[:])
                fixo_sb = cpool.tile([1, QL], F16, tag="fixo")
                nc.gpsimd.dma_start(out=fixo_sb[:], in_=fixod[:])
                fixs_sb = cpool.tile([1, QL], F16, tag="fixs")
                nc.gpsimd.dma_start(out=fixs_sb[:], in_=fixsd[:])
                return b_sb, mask_sb, zero_sb, ones_sb, iden_sb, fixo_sb, fixs_sb

            # ---- persistent buffers ----
            kT_sb = kvpool.tile([DK, S], F16, tag="kT")
            qT_sb = kvpool.tile([DK, QL], F16, tag="qT")
            vT_sb = kvpool.tile([DK, S], F16, tag="vT")
            vnat_sb = kvpool.tile([128, NCHUNK, DK], F16, tag="vnat")

            ps_out = ps_out_pool.tile([DK, QL], F32)       # 2 banks
            vred4 = opool.tile([DK, NBLK], F32, tag="vred4")
            ps_sums = ps_sums_pool.tile([1, QL], F32)      # 2 banks
            nc.vector.memset(ps_out[:], 0.0)
            nc.vector.memset(ps_sums[:], 0.0)

            for blk in range(NBLK):
                s0 = blk * 512
                # ---- stream X^T block: 8 d-chunk tiles x 512 s-cols ----
                xb = xpool.tile([128, 8, 512], F16, tag="xb")
                for dc in range(4):
                    nc.sync.dma_start(
                        out=xb[:, 2 * dc:2 * dc + 2],
                        in_=xt[256 * dc:256 * dc + 256, s0:s0 + 512]
                        .rearrange("(i p) s -> p i s", p=128),
                    )
                if blk == 0:
                    (b_sb, mask_sb, zero_sb, ones_sb, iden_sb,
                     fixo_sb, fixs_sb) = small_consts()
                    scratch = cpool.tile([1, 1], F32, tag="scratch")
                    nc.scalar.activation(scratch[:], mask_sb[0:1, 0:1], AF.Ln)

                # ---- K^T / V^T projections for this block ----
                for name, dst in (("k", kT_sb), ("v", vT_sb)):
                    pp = ps_proj_pool.tile([DK, 512], F32, tag="pp")
                    for dc in range(8):
                        nc.tensor.matmul(
                            pp[:], w_sb[name][:, dc], xb[:, dc],
                            start=(dc == 0), stop=(dc == 7),
                        )
                    nc.vector.tensor_scalar_add(
                        dst[:, s0:s0 + 512], pp[:], b_sb[name][:],
                    )

                # ---- Q^T projection: first 64 cols of each 128-tile ----
                pq = ps_proj_pool.tile([DK, 256], F32, tag="pp")
                for dc in range(8):
                    qmov = xb[:, dc].rearrange("p (t j) -> p t j", t=4)[:, :, 0:64]
                    nc.tensor.matmul(
                        pq[:], w_sb["q"][:, dc], qmov,
                        start=(dc == 0), stop=(dc == 7),
                    )
                q0 = blk * 256
                nc.vector.tensor_scalar_add(qT_sb[:, q0:q0 + 256], pq[:], b_sb["q"][:])

                nc.vector.tensor_reduce(
                    vred4[:, blk:blk + 1], vT_sb[:, s0:s0 + 512],
                    mybir.AxisListType.X, mybir.AluOpType.add,
                )

                # ---- V natural tiles (transpose V^T chunks) ----
                tp4 = ps_proj_pool.tile([128, 4, 128], F16, tag="pp")
                for t in range(4):
                    c = 4 * blk + t
                    nc.tensor.matmul(
                        tp4[:, t], vT_sb[:, 128 * c:128 * c + 128], iden_sb[:],
                        is_transpose=True, start=(t == 0), stop=(t == 3),
                    )
                nc.vector.tensor_copy(vnat_sb[:, 4 * blk:4 * blk + 4], tp4[:])
                if blk == NBLK - 1:
                    # Vsum chain: emitted here (deps ready) so the PE's
                    # in-order stream handles the tiny transpose mid-flight
                    vred = opool.tile([DK, 1], F32, tag="vred")
                    nc.vector.tensor_reduce(vred[:], vred4[:],
                                            mybir.AxisListType.X,
                                            mybir.AluOpType.add)
                    vredr = opool.tile([DK, 1], F16, tag="vredr")
                    nc.vector.tensor_copy(vredr[:], vred[:])
                    vs = ps_proj_pool.tile([1, DK], F16, tag="pp")
                    nc.tensor.matmul(vs[:], vredr[:], iden_sb[:],
                                     is_transpose=True, start=True, stop=True)
                    vsT_sb = opool.tile([1, DK], F16, tag="vsT")
                    nc.vector.tensor_copy(vsT_sb[:], vs[:])

                # ---- attention chunks for this block ----
                # last block reversed: the big chunk 15 goes first so the
                # serial Vsum/fix/normalize chain overlaps the small chunks
                order = range(3, -1, -1) if blk == NBLK - 1 else range(4)
                for t in order:
                    c = 4 * blk + t
                    prefix = 64 * (c + 1)
                    dcol = 64 * c  # diagonal columns [dcol, dcol+64)
                    pieces = [(p, min(512, prefix - p))
                              for p in range(0, prefix, 512)]
                    kT_c = kT_sb[:, 128 * c:128 * c + 128]
                    for (p0, pn) in pieces:
                        mn = pn
                        sc = ps_score_pool.tile([128, 512], F32, tag="sc")
                        nc.tensor.matmul(
                            sc[:, 0:mn], kT_c, qT_sb[:, p0:p0 + mn],
                            start=True, stop=True,
                        )
                        if p0 <= dcol < p0 + pn:
                            dl = dcol - p0
                            nc.vector.tensor_tensor(
                                sc[:, dl:dl + 64], sc[:, dl:dl + 64],
                                mask_sb[:], mybir.AluOpType.add,
                            )
                        pt = ppool.tile([128, 512], F16, tag="pt")
                        nc.scalar.activation(pt[:, 0:pn], sc[:, 0:pn], AF.Exp)
                        if mn > pn:
                            nc.vector.tensor_copy(pt[:, pn:mn],
                                                  zero_sb[:, pn:mn])
                        # the accumulators were DVE-zeroed once up front, so
                        # every matmul accumulates (start=False)
                        nc.tensor.matmul(
                            ps_out[:, p0:p0 + mn], vnat_sb[:, c],
                            pt[:, 0:mn], start=False, stop=False,
                        )
                        nc.tensor.matmul(
                            ps_sums[:, p0:p0 + mn], ones_sb[:, 0:1],
                            pt[:, 0:mn], start=False, stop=False,
                        )

            # ---- fix for the globally fully-masked last row ----
            for p0 in (0, 512):
                nc.tensor.matmul(ps_out[:, p0:p0 + 512], vsT_sb[:],
                                 fixo_sb[:, p0:p0 + 512], start=False, stop=True)
                nc.tensor.matmul(ps_sums[:, p0:p0 + 512], ones_sb[0:1, 0:1],
                                 fixs_sb[:, p0:p0 + 512], start=False, stop=True)

            # ---- normalize and store ----
            lns_sb = opool.tile([1, QL], F32, tag="lns")
            nc.scalar.activation(lns_sb[:], ps_sums[:], AF.Ln)
            recip_sb = opool.tile([1, QL], F16, tag="recip")
            nc.scalar.activation(recip_sb[:], lns_sb[:], AF.Exp, scale=-1.0)
            o_sb = opool.tile([DK, QL], F32, tag="o")
            for p0 in (0, 512):
                rb = ps_score_pool.tile([128, 512], F32, tag="sc")
                nc.tensor.matmul(rb[:], ones_sb[0:1, :],
                                 recip_sb[:, p0:p0 + 512], start=True, stop=True)
                rb_sb = opool.tile([128, 512], F32, tag="rb")
                nc.scalar.activation(rb_sb[:], rb[:], AF.Identity)
                nc.vector.tensor_tensor(o_sb[:, p0:p0 + 512],
                                        ps_out[:, p0:p0 + 512], rb_sb[:],
                                        mybir.AluOpType.mult)
            nc.sync.dma_start(out=outT[:], in_=o_sb[:])

    nc.compile()
    return nc


def _prep_inputs(inputs, Wq, bq, Wk, bk, Wv, bv):
    scale = np.float32(1.0 / np.sqrt(DK))
    wq_s = np.ascontiguousarray((Wq * scale).reshape(8, 128, DK).transpose(1, 0, 2)).astype(np.float16)
    wk_s = np.ascontiguousarray(Wk.reshape(8, 128, DK).transpose(1, 0, 2)).astype(np.float16)
    wv_s = np.ascontiguousarray(Wv.reshape(8, 128, DK).transpose(1, 0, 2)).astype(np.float16)
    bq_s = np.ascontiguousarray((bq * scale).reshape(DK, 1), dtype=np.float32)
    bk_s = np.ascontiguousarray(bk.reshape(DK, 1), dtype=np.float32)
    bv_s = np.ascontiguousarray(bv.reshape(DK, 1), dtype=np.float32)
    ones = np.ones((128, 128), dtype=np.float16)
    iden = np.eye(128, dtype=np.float16)

    p = np.arange(128)[:, None]
    j = np.arange(64)[None, :]
    masks = []
    for h in (0, 1):
        m = np.zeros((128, 64), dtype=np.float32)
        m[(p < 64) & (p <= j)] = NEG
        if h == 1:
            m[p[:, 0] >= 64, :] = NEG
        masks.append(m)

    in_maps = []
    for core in range(NCORES):
        b, h = core // 2, core % 2
        xt = inputs[b].T.reshape(D, 16, 2, 64)
        if h == 1:
            xt = xt[:, :, ::-1, :]
        xt = np.ascontiguousarray(xt).reshape(D, S).astype(np.float16)
        fixo = np.zeros((1, QL), dtype=np.float16)
        fixs = np.zeros((1, QL), dtype=np.float16)
        if h == 1:
            fixo[0, QL - 1] = 1.0 / S
            fixs[0, QL - 1] = 1.0
        in_maps.append({
            "xt": xt, "wq": wq_s, "wk": wk_s, "wv": wv_s,
            "bq": bq_s, "bk": bk_s, "bv": bv_s,
            "maskd": masks[h], "onesd": ones, "idend": iden,
            "zerod": np.zeros((128, 512), dtype=np.float16),
            "fixod": fixo, "fixsd": fixs,
        })
    return in_maps


def kernel(inputs, Wq, bq, Wk, bk, Wv, bv):
    inputs = np.asarray(inputs, dtype=np.float32)
    if "nc" not in _cache:
        _cache["nc"] = _build()
    nc = _cache["nc"]
    in_maps = _prep_inputs(inputs, np.asarray(Wq), np.asarray(bq),
                           np.asarray(Wk), np.asarray(bk),
                           np.asarray(Wv), np.asarray(bv))
    res = run_bass_kernel_spmd(nc, in_maps, list(range(NCORES)))
    out = np.empty((B, S, DK), dtype=np.float32)
    for core in range(NCORES):
        b, h = core // 2, core % 2
        oT = res.results[core]["outT"]          # [DK, 1024], cols = (c, j)
        o = oT.T.reshape(16, 64, DK)            # [c, j, DK]
        out[b].reshape(16, 2, 64, DK)[:, h] = o
    return out



# revision 12
# speedup vs baseline: 1.0272x; 1.0272x over previous
"""Masked self-attention Trainium2 kernel (8 NeuronCores, Bass/Tile).

Problem: B=4, S=2048, D=1024, DK=128 fp32.
  Q = X@Wq + bq; K = X@Wk + bk; V = X@Wv + bv
  scores = Q@K^T / sqrt(DK); masked = scores + tril(ones)*(-1e9)
  out = softmax(masked) @ V

Sharding: core = (batch b = core//2) x (row-half h = core%2). Each core
computes 64 query rows of each of the 16 query tiles of its batch
(rows 128c + 64h + j). All cores run an identical program; per-core
differences are carried entirely in the input data (a column
permutation of X^T and a small mask block).

Device layouts (all transposed so the PE contracts over partitions):
  X^T [D, S] (host-transposed, per-tile column permuted: own rows first),
  streamed in 2 superblocks of 1024 columns.
  Q^T/K^T/V^T [DK, *] = W-chunks(lhsT) x X^T(moving) fp16 matmuls with
  fp32 PSUM accumulation; Q projected only for the core's own 64-column
  tile halves (strided moving AP), 512 columns per superblock.
  scores^T [s-chunk 128, q-cols] = K^T-chunk(lhsT) x Q^T(moving)
  causal skip: chunk c only attends query tiles qi <= c -> contiguous
  q-prefix of width 64*(c+1); single [128,64] mask block on the
  diagonal tile. The -1e9 add absorbs the score entirely in fp32
  (|score| << ulp(1e9)), matching the reference bit pattern, and exp
  underflows masked lanes to exactly 0.
  softmax: exp without max-subtraction (scores are O(1)); row sums
  accumulated on the DVE into an fp16 staging tile (sacc += pt per
  piece) and reduced once per 512-column half with an M=1 all-ones
  matmul -- this removes a third matmul pass over every attention
  piece. Normalization via DVE reciprocal_approx_fast (no Ln/Exp
  activation-table reloads; the scalar engine only ever loads the Exp
  table) and an f32r K=1 matmul broadcast of 1/sums across partitions.
  out^T [DK, 1024] accumulated in PSUM across s-chunks.

  Attention runs in two phases: phase A covers query columns
  [512,1024) (chunks 15..8), phase B covers [0,512) (chunks 15..0).
  The upper output half is summed/normalized/DMA'd while phase B still
  streams on the PE, so only the lower half's normalize chain remains
  on the critical tail. The globally fully-masked last row (2047) is
  patched on the host with mean(V) -- exactly what the fp32 reference
  computes for it, since scores - 1e9 == -1e9 in fp32 makes its
  softmax uniform. Its on-device column underflows to 0/0 = NaN and is
  overwritten.

  All matmul operands are float16 (11-bit mantissa, ~2.4e-4 rounding)
  with fp32 PSUM accumulation; the output is stored fp16 (halves the
  store DMA) and widened on the host. The first weight chunk gets a
  dedicated small first-wave DMA because the DGE queues fair-share HBM
  bandwidth and gate the first matmul.
"""

import numpy as np

import concourse.bacc as bacc
import concourse.tile as tile
import concourse.mybir as mybir
from concourse.bass_utils import run_bass_kernel_spmd

F32 = mybir.dt.float32
F32R = mybir.dt.float32r
F16 = mybir.dt.float16
AF = mybir.ActivationFunctionType

B, S, D, DK = 4, 2048, 1024, 128
NEG = -1.0e9
NCORES = 8
NSB = 2           # superblocks of 1024 s-columns
NCHUNK = 16       # s-chunks of 128
QL = 1024         # local query columns per core (16 tiles x 64)

_cache = {}


def _build():
    nc = bacc.Bacc("TRN2", target_bir_lowering=False, debug=False,
                   num_devices=NCORES)

    xt = nc.dram_tensor("xt", [D, S], F16, kind="ExternalInput")
    wq = nc.dram_tensor("wq", [128, 8, DK], F16, kind="ExternalInput")
    wk = nc.dram_tensor("wk", [128, 8, DK], F16, kind="ExternalInput")
    wv = nc.dram_tensor("wv", [128, 8, DK], F16, kind="ExternalInput")
    bq = nc.dram_tensor("bq", [DK, 1], F32, kind="ExternalInput")
    bk = nc.dram_tensor("bk", [DK, 1], F32, kind="ExternalInput")
    bv = nc.dram_tensor("bv", [DK, 1], F32, kind="ExternalInput")
    maskd = nc.dram_tensor("maskd", [128, 64], F32, kind="ExternalInput")
    onesd = nc.dram_tensor("onesd", [128, 1], F16, kind="ExternalInput")
    idend = nc.dram_tensor("idend", [128, 128], F16, kind="ExternalInput")
    onesfd = nc.dram_tensor("onesfd", [1, 128], F16, kind="ExternalInput")
    outT = nc.dram_tensor("outT", [DK, QL], F16, kind="ExternalOutput")

    with tile.TileContext(nc) as tc:
        with (
            tc.tile_pool(name="consts", bufs=1) as cpool,
            tc.tile_pool(name="xblk", bufs=2) as xpool,
            tc.tile_pool(name="kv", bufs=1) as kvpool,
            tc.tile_pool(name="pt", bufs=3) as ppool,
            tc.tile_pool(name="outp", bufs=1) as opool,
            tc.tile_pool(name="ps_out", bufs=1, space="PSUM") as ps_out_pool,
            tc.tile_pool(name="ps_proj", bufs=2, space="PSUM") as ps_proj_pool,
            tc.tile_pool(name="ps_score", bufs=2, space="PSUM") as ps_score_pool,
            tc.tile_pool(name="ps_rb", bufs=2, space="PSUM") as ps_rb_pool,
        ):
            # ---- weights first (needed by the very first matmul).
            # The first proj matmul (K, dc=0) gates the whole PE stream, so
            # its weight chunk gets a dedicated small first-wave DMA: the DGE
            # queues fair-share HBM bandwidth, so a small exclusive first
            # wave completes far sooner than one queued with everything.
            w_sb = {}
            for name, dram in (("k", wk), ("v", wv), ("q", wq)):
                t = cpool.tile([128, 8, DK], F16, tag=f"w{name}")
                if name == "k":
                    nc.scalar.dma_start(out=t[:, 0:1], in_=dram[:, 0:1])
                    nc.scalar.dma_start(out=t[:, 1:8], in_=dram[:, 1:8])
                else:
                    nc.scalar.dma_start(out=t[:], in_=dram[:])
                w_sb[name] = t

            def small_consts():
                b_sb = {}
                for name, dram in (("q", bq), ("k", bk), ("v", bv)):
                    t = cpool.tile([DK, 1], F32, tag=f"b{name}")
                    nc.gpsimd.dma_start(out=t[:], in_=dram[:])
                    b_sb[name] = t
                mask_sb = cpool.tile([128, 64], F32, tag="mask")
                nc.gpsimd.dma_start(out=mask_sb[:], in_=maskd[:])
                ones_sb = cpool.tile([128, 1], F16, tag="ones")
                nc.gpsimd.dma_start(out=ones_sb[:], in_=onesd[:])
                iden_sb = cpool.tile([128, 128], F16, tag="iden")
                nc.gpsimd.dma_start(out=iden_sb[:], in_=idend[:])
                onesf_sb = cpool.tile([1, 128], F16, tag="onesf")
                nc.gpsimd.dma_start(out=onesf_sb[:], in_=onesfd[:])
                return b_sb, mask_sb, ones_sb, iden_sb, onesf_sb

            # ---- persistent buffers ----
            kT_sb = kvpool.tile([DK, S], F16, tag="kT")
            vT_sb = kvpool.tile([DK, S], F16, tag="vT")
            qT_sb = kvpool.tile([DK, QL], F16, tag="qT")
            vnat_sb = kvpool.tile([128, NCHUNK, DK], F16, tag="vnat")
            sacc_sb = kvpool.tile([128, QL], F16, tag="sacc")
            o_sb = opool.tile([DK, QL], F16, tag="o")
            recip_u = opool.tile([1, 512], F32, tag="recip_u")
            recip_l = opool.tile([1, 512], F32, tag="recip_l")
            recip_u16 = opool.tile([1, 512], F16, tag="recip_u16")
            recip_l16 = opool.tile([1, 512], F16, tag="recip_l16")
            rbu_sb = opool.tile([128, 512], F32, tag="rbu")
            rbl_sb = opool.tile([128, 512], F32, tag="rbl")

            ps_out = ps_out_pool.tile([DK, QL], F32)       # 2 banks
            nc.vector.memset(ps_out[:], 0.0)
            nc.vector.memset(sacc_sb[:], 0.0)

            # ---- projections, 2 superblocks of 1024 columns ----
            for sb in range(NSB):
                s0 = 1024 * sb
                xb = xpool.tile([128, 8, 1024], F16, tag="xb")
                nc.sync.dma_start(
                    out=xb[:, 0:2, 0:512],
                    in_=xt[0:256, s0:s0 + 512]
                    .rearrange("(i p) s -> p i s", p=128))
                nc.sync.dma_start(
                    out=xb[:, 2:8, 0:512],
                    in_=xt[256:1024, s0:s0 + 512]
                    .rearrange("(i p) s -> p i s", p=128))
                nc.sync.dma_start(
                    out=xb[:, :, 512:1024],
                    in_=xt[:, s0 + 512:s0 + 1024]
                    .rearrange("(i p) s -> p i s", p=128))
                if sb == 0:
                    b_sb, mask_sb, ones_sb, iden_sb, onesf_sb = small_consts()

                for half in range(2):
                    cl = slice(s0 + 512 * half, s0 + 512 * half + 512)
                    xl = slice(512 * half, 512 * half + 512)
                    for name, dst in (("k", kT_sb), ("v", vT_sb)):
                        pp = ps_proj_pool.tile([DK, 512], F32, tag="pp")
                        for dc in range(8):
                            nc.tensor.matmul(
                                pp[:], w_sb[name][:, dc], xb[:, dc, xl],
                                start=(dc == 0), stop=(dc == 7))
                        nc.vector.tensor_scalar_add(dst[:, cl], pp[:],
                                                    b_sb[name][:])

                # Q: first 64 cols of each 128-tile (own queries)
                pq = ps_proj_pool.tile([DK, 512], F32, tag="pp")
                for dc in range(8):
                    qmov = (xb[:, dc].rearrange("p (t j) -> p t j", t=8)
                            [:, :, 0:64])
                    nc.tensor.matmul(pq[:], w_sb["q"][:, dc], qmov,
                                     start=(dc == 0), stop=(dc == 7))
                q0 = 512 * sb
                nc.vector.tensor_scalar_add(qT_sb[:, q0:q0 + 512], pq[:],
                                            b_sb["q"][:])

                # V natural tiles (transpose V^T chunks)
                for g in range(2):
                    tp = ps_proj_pool.tile([128, 4, 128], F16, tag="pp")
                    for t in range(4):
                        c = 8 * sb + 4 * g + t
                        nc.tensor.matmul(
                            tp[:, t], vT_sb[:, 128 * c:128 * c + 128],
                            iden_sb[:], is_transpose=True,
                            start=(t == 0), stop=(t == 3))
                    c0 = 8 * sb + 4 * g
                    nc.vector.tensor_copy(vnat_sb[:, c0:c0 + 4], tp[:])

            # ---- attention pieces ----
            def attn_piece(c, p0, pn, stop=False):
                kT_c = kT_sb[:, 128 * c:128 * c + 128]
                sc = ps_score_pool.tile([128, 512], F32, tag="sc")
                nc.tensor.matmul(sc[:, 0:pn], kT_c, qT_sb[:, p0:p0 + pn],
                                 start=True, stop=True)
                dcol = 64 * c
                if p0 <= dcol < p0 + pn:
                    dl = dcol - p0
                    nc.vector.tensor_tensor(
                        sc[:, dl:dl + 64], sc[:, dl:dl + 64],
                        mask_sb[:], mybir.AluOpType.add)
                pt = ppool.tile([128, 512], F16, tag="pt")
                nc.scalar.activation(pt[:, 0:pn], sc[:, 0:pn], AF.Exp)
                nc.tensor.matmul(ps_out[:, p0:p0 + pn], vnat_sb[:, c],
                                 pt[:, 0:pn], start=False, stop=stop)
                nc.vector.tensor_tensor(
                    sacc_sb[:, p0:p0 + pn], sacc_sb[:, p0:p0 + pn],
                    pt[:, 0:pn], mybir.AluOpType.add)

            # Phase A: query columns [512, 1024) -- chunks 15..8
            for c in range(15, 7, -1):
                attn_piece(c, 512, 64 * (c + 1) - 512, stop=(c == 8))

            # upper half row sums (waits on phase A's sacc adds)
            sums_u = ps_rb_pool.tile([1, 512], F32, tag="rbsum")
            nc.tensor.matmul(sums_u[:], ones_sb[:, 0:1],
                             sacc_sb[:, 512:1024], start=True, stop=True)

            # Phase B part 1: chunks 15..8, columns [0, 512)
            for c in range(15, 7, -1):
                attn_piece(c, 0, 512)

            # upper finalize (overlaps phase B on PE)
            nc.vector.reciprocal_approx_fast(recip_u[:], sums_u[:])
            nc.vector.tensor_copy(recip_u16[:], recip_u[:])
            rb_u = ps_rb_pool.tile([128, 512], F32, tag="rbsum")
            nc.tensor.matmul(rb_u[:], onesf_sb[:], recip_u16[:],
                             start=True, stop=True)

            # Phase B part 2: chunks 7..0, columns [0, 64*(c+1)).
            # The upper-half multiply + store are emitted two pieces in so
            # the DVE mask adds of the first B2 pieces are not delayed.
            for c in range(7, -1, -1):
                attn_piece(c, 0, 64 * (c + 1), stop=(c == 0))
                if c == 6:
                    nc.vector.tensor_copy(rbu_sb[:], rb_u[:])
                    nc.vector.tensor_tensor(
                        o_sb[:, 512:1024], ps_out[:, 512:1024],
                        rbu_sb[:], mybir.AluOpType.mult)
                    nc.gpsimd.dma_start(out=outT[:, 512:1024],
                                        in_=o_sb[:, 512:1024])

            # lower finalize
            sums_l = ps_rb_pool.tile([1, 512], F32, tag="rbsum")
            nc.tensor.matmul(sums_l[:], ones_sb[:, 0:1], sacc_sb[:, 0:512],
                             start=True, stop=True)
            nc.vector.reciprocal_approx_fast(recip_l[:], sums_l[:])
            nc.vector.tensor_copy(recip_l16[:], recip_l[:])
            rb_l = ps_rb_pool.tile([128, 512], F32, tag="rbsum")
            nc.tensor.matmul(rb_l[:], onesf_sb[:], recip_l16[:],
                             start=True, stop=True)
            nc.vector.tensor_copy(rbl_sb[:], rb_l[:])
            nc.vector.tensor_tensor(o_sb[:, 0:512], ps_out[:, 0:512],
                                    rbl_sb[:], mybir.AluOpType.mult)
            nc.sync.dma_start(out=outT[:, 0:512], in_=o_sb[:, 0:512])

    nc.compile()
    return nc


def _prep_inputs(inputs, Wq, bq, Wk, bk, Wv, bv):
    scale = np.float32(1.0 / np.sqrt(DK))
    wq_s = np.ascontiguousarray((Wq * scale).reshape(8, 128, DK).transpose(1, 0, 2)).astype(np.float16)
    wk_s = np.ascontiguousarray(Wk.reshape(8, 128, DK).transpose(1, 0, 2)).astype(np.float16)
    wv_s = np.ascontiguousarray(Wv.reshape(8, 128, DK).transpose(1, 0, 2)).astype(np.float16)
    bq_s = np.ascontiguousarray((bq * scale).reshape(DK, 1), dtype=np.float32)
    bk_s = np.ascontiguousarray(bk.reshape(DK, 1), dtype=np.float32)
    bv_s = np.ascontiguousarray(bv.reshape(DK, 1), dtype=np.float32)
    ones = np.ones((128, 1), dtype=np.float16)
    iden = np.eye(128, dtype=np.float16)
    onesf = np.ones((1, 128), dtype=np.float16)

    p = np.arange(128)[:, None]
    j = np.arange(64)[None, :]
    masks = []
    for h in (0, 1):
        m = np.zeros((128, 64), dtype=np.float32)
        m[(p < 64) & (p <= j)] = NEG
        if h == 1:
            m[p[:, 0] >= 64, :] = NEG
        masks.append(m)

    in_maps = []
    for core in range(NCORES):
        b, h = core // 2, core % 2
        xtc = inputs[b].T.reshape(D, 16, 2, 64)
        if h == 1:
            xtc = xtc[:, :, ::-1, :]
        xtc = np.ascontiguousarray(xtc).reshape(D, S).astype(np.float16)
        in_maps.append({
            "xt": xtc, "wq": wq_s, "wk": wk_s, "wv": wv_s,
            "bq": bq_s, "bk": bk_s, "bv": bv_s,
            "maskd": masks[h], "onesd": ones, "idend": iden,
            "onesfd": onesf,
        })
    return in_maps


def kernel(inputs, Wq, bq, Wk, bk, Wv, bv):
    inputs = np.asarray(inputs, dtype=np.float32)
    Wq, bq = np.asarray(Wq), np.asarray(bq)
    Wk, bk = np.asarray(Wk), np.asarray(bk)
    Wv, bv = np.asarray(Wv), np.asarray(bv)
    if "nc" not in _cache:
        _cache["nc"] = _build()
    nc = _cache["nc"]
    in_maps = _prep_inputs(inputs, Wq, bq, Wk, bk, Wv, bv)
    res = run_bass_kernel_spmd(nc, in_maps, list(range(NCORES)))
    out = np.empty((B, S, DK), dtype=np.float32)
    for core in range(NCORES):
        b, h = core // 2, core % 2
        oT = res.results[core]["outT"].astype(np.float32)  # [DK, 1024]
        o = oT.T.reshape(16, 64, DK)                       # [c, j, DK]
        out[b].reshape(16, 2, 64, DK)[:, h] = o
    # Row 2047 is fully masked: scores - 1e9 == -1e9 exactly in fp32, so
    # the reference's softmax over it is uniform -> mean(V). On device it
    # underflows to 0/0; patch it here.
    meanV = inputs.mean(axis=1) @ Wv + bv                  # [B, DK]
    out[:, S - 1, :] = meanV
    return out
